# revision 9
# baseline (speedup 1.0000x reference)
# Trainium2 Bass kernel for nn_BasicBlock (FISTA sparse-coding BasicBlock).
#
# Data-parallel over batch: 32 samples -> 8 NeuronCores x 4 samples.
# Per sample, everything stays in SBUF; convs are 9-tap accumulating PE
# matmuls (channel blocks of 128 on partitions, 392-pixel N tiles);
# conv-transpose is computed via output-parity-class gather (9 taps total
# across 4 classes -> no zero-insertion waste). FISTA elementwise work runs
# on DVE/ACT. Dictionary normalization, MU scaling, weight transposes and
# BN folds are host-side numpy prep.
#
# Self-contained: hardcodes shapes from the problem spec.
import os
import sys
import time

sys.path.insert(0, "/opt/trn_rl_repo")

import numpy as np
import ml_dtypes

import concourse.bass as bass  # noqa: F401  (bass types referenced via bacc)
import concourse.mybir as mybir
from concourse import bacc
from concourse.bass_utils import run_bass_kernel_spmd
from concourse.tile import TileContext
from contextlib import ExitStack

F32 = mybir.dt.float32
BF16 = mybir.dt.bfloat16

MU = 0.1
LMBD = 0.1
N_STEPS = 4
BN_EPS = 1e-5
N_CORES = 8
NS = 4  # samples per core

RELU = mybir.ActivationFunctionType.Relu
IDENT = mybir.ActivationFunctionType.Identity

# FISTA momentum coefficients (matches reference's python-float t sequence)
BETAS = []
_t = 1.0
for _ in range(N_STEPS - 1):
    _tn = (1.0 + float(np.sqrt(1.0 + 4.0 * _t * _t))) / 2.0
    BETAS.append((_t - 1.0) / _tn)
    _t = _tn

# conv_t stride-2 parity classes: (ey, ex) -> [(dy, dx, du, dv)]
CT_CLASSES = [
    ((0, 0), [(1, 1, 0, 0)]),
    ((0, 1), [(1, 0, 0, 1), (1, 2, 0, 0)]),
    ((1, 0), [(0, 1, 1, 0), (2, 1, 0, 0)]),
    ((1, 1), [(0, 0, 1, 1), (0, 2, 1, 0), (2, 0, 0, 1), (2, 2, 0, 0)]),
]

KERNEL_STATS = {}
_PROGRAM_CACHE = {}


def _build_program(cdt):
    """Build + compile the per-core Bass program. cdt: matmul operand dtype."""
    nc = bacc.Bacc("TRN2", num_devices=1, debug=False)

    x_d = nc.dram_tensor("x", [NS, 128, 56, 56], cdt, kind="ExternalInput")
    w1f_d = nc.dram_tensor("w1f", [128, 9, 256], cdt, kind="ExternalInput")
    w1t_d = nc.dram_tensor("w1t", [128, 2, 9, 128], cdt, kind="ExternalInput")
    w2f_d = nc.dram_tensor("w2f", [128, 2, 9, 256], cdt, kind="ExternalInput")
    w2t_d = nc.dram_tensor("w2t", [128, 2, 9, 256], cdt, kind="ExternalInput")
    wsc_d = nc.dram_tensor("wsc", [128, 256], cdt, kind="ExternalInput")
    bn_d = {}
    for nm in ("bn1s", "bn1t", "bn2s", "bn2t", "bnscs", "bnsct"):
        bn_d[nm] = nc.dram_tensor(nm, [128, 2], F32, kind="ExternalInput")
    out_d = nc.dram_tensor("out", [NS, 256, 28, 28], F32, kind="ExternalOutput")

    with TileContext(nc) as tc:
        with ExitStack() as es:
            consts = es.enter_context(tc.tile_pool(name="consts", bufs=1))
            state = es.enter_context(tc.tile_pool(name="state", bufs=1))
            xin = es.enter_context(tc.tile_pool(name="xin", bufs=2))
            outp = es.enter_context(tc.tile_pool(name="outp", bufs=2))
            psum = es.enter_context(tc.tile_pool(name="psum", bufs=8, space="PSUM"))

            # ---- constants ----
            w1f = consts.tile([128, 9, 256], cdt)
            w1t = consts.tile([128, 2, 9, 128], cdt)
            w2f = consts.tile([128, 2, 9, 256], cdt)
            w2t = consts.tile([128, 2, 9, 256], cdt)
            wsc = consts.tile([128, 256], cdt)
            nc.sync.dma_start(out=w1f[:], in_=w1f_d.ap())
            nc.sync.dma_start(out=w1t[:], in_=w1t_d.ap())
            nc.sync.dma_start(out=w2f[:], in_=w2f_d.ap())
            nc.sync.dma_start(out=w2t[:], in_=w2t_d.ap())
            nc.sync.dma_start(out=wsc[:], in_=wsc_d.ap())
            bn = {}
            for nm in bn_d:
                bn[nm] = consts.tile([128, 2], F32, name=nm)
                nc.sync.dma_start(out=bn[nm][:], in_=bn_d[nm].ap())
            neg_thr = consts.tile([128, 1], F32)
            nc.vector.memset(neg_thr[:], -LMBD * MU)

            # ---- persistent per-sample state (interiors rewritten per
            # sample; borders stay zero from this one-time memset) ----
            r1 = state.tile([128, 58, 58], cdt)
            a1 = state.tile([128, 2, 29, 29], cdt)
            c1A = state.tile([128, 2, 29, 29], F32)
            c1B = state.tile([128, 2, 29, 29], F32)
            x2 = state.tile([128, 2, 30, 30], cdt)
            r2 = state.tile([128, 2, 30, 30], cdt)
            a2 = state.tile([128, 2, 30, 30], cdt)
            c2A = state.tile([128, 2, 30, 30], F32)
            c2B = state.tile([128, 2, 30, 30], F32)
            hb = state.tile([128, 2, 784], F32)
            dtmp = state.tile([128, 2, 28, 29], F32)
            for t_ in (r1, a1, c1A, c1B, x2, r2, a2, c2A, c2B):
                nc.gpsimd.memset(t_[:], 0.0)
            # fence: all init DMAs/memsets complete before any compute
            # (first-sample data race observed on HW without this)
            tc.strict_bb_all_engine_barrier()

            def ps_tile():
                return psum.tile([128, 392], F32, name="pt", tag="ps")

            def mm(pt, lhsT, rhs, start, stop):
                nc.tensor.matmul(
                    pt[:].rearrange("p (u v) -> p u v", v=28),
                    lhsT, rhs, start=start, stop=stop)

            # ---- conv emitters (all produce 4 or 8 psum tiles, then run
            # `consume(cb_or_class, h, pt)`) ----
            def conv1_fwd(src, consume):
                # stride-2 3x3 conv, 128ci -> 256co, src [128,58,58] padded
                for cb in range(2):
                    for h in range(2):
                        pt = ps_tile()
                        for tap in range(9):
                            dy, dx = tap // 3, tap % 3
                            rhs = src[:, 28 * h + dy: 28 * h + dy + 28: 2,
                                      dx: dx + 56: 2]
                            mm(pt, w1f[:, tap, cb * 128:(cb + 1) * 128], rhs,
                               tap == 0, tap == 8)
                        consume(cb, h, pt)

            def conv1_t(src, consume):
                # stride-2 conv-transpose, 256 -> 128, src [128,2,29,29]
                for (ey, ex), taps in CT_CLASSES:
                    for h in range(2):
                        pt = ps_tile()
                        n = 2 * len(taps)
                        i = 0
                        for (dy, dx, du, dv) in taps:
                            tap = dy * 3 + dx
                            for kb in range(2):
                                rhs = src[:, kb, du + 14 * h: du + 14 * h + 14,
                                          dv: dv + 28]
                                mm(pt, w1t[:, kb, tap, :], rhs,
                                   i == 0, i == n - 1)
                                i += 1
                        consume((ey, ex), h, pt)

            def conv2_fwd(src, consume):
                # stride-1 3x3 conv, 256 -> 256, src [128,2,30,30] padded
                for cb in range(2):
                    for h in range(2):
                        pt = ps_tile()
                        for i in range(18):
                            kb, tap = i // 9, i % 9
                            dy, dx = tap // 3, tap % 3
                            rhs = src[:, kb, 14 * h + dy: 14 * h + dy + 14,
                                      dx: dx + 28]
                            mm(pt, w2f[:, kb, tap, cb * 128:(cb + 1) * 128],
                               rhs, i == 0, i == 17)
                        consume(cb, h, pt)

            def conv2_t(src, consume):
                # stride-1 conv-transpose (flipped taps), src [128,2,30,30]
                for cb in range(2):
                    for h in range(2):
                        pt = ps_tile()
                        for i in range(18):
                            kb, tap = i // 9, i % 9
                            dy, dx = tap // 3, tap % 3
                            rhs = src[:, kb, (2 - dy) + 14 * h:
                                      (2 - dy) + 14 * h + 14,
                                      (2 - dx): (2 - dx) + 28]
                            mm(pt, w2t[:, kb, tap, cb * 128:(cb + 1) * 128],
                               rhs, i == 0, i == 17)
                        consume(cb, h, pt)

            def int1(c):  # interior of a block-1 c tile ([128,2,28,28])
                return c[:, :, 0:28, 0:28]

            def int2(c):  # interior of a block-2 30x30 tile
                return c[:, :, 1:29, 1:29]

            def relu_into(c_t, interior):
                # c = relu(c - lmbd*mu) in place over the interior view
                nc.scalar.activation(interior, interior, RELU, bias=neg_thr[:])
                _ = c_t  # (kept for readability at call sites)

            # ================= per-sample program =================
            for s in range(NS):
                x_pad = xin.tile([128, 58, 58], cdt, name="x_pad", tag="xpad")
                # borders zero, interior from DRAM
                nc.gpsimd.memset(x_pad[:, 0, :], 0.0)
                nc.gpsimd.memset(x_pad[:, 57, :], 0.0)
                nc.gpsimd.memset(x_pad[:, 1:57, 0], 0.0)
                nc.gpsimd.memset(x_pad[:, 1:57, 57], 0.0)
                nc.sync.dma_start(out=x_pad[:, 1:57, 1:57], in_=x_d.ap()[s])

                # ---------- block 1 (stride-2 FISTA on x) ----------
                def c1_init(cb, h, pt):
                    nc.scalar.activation(
                        c1A[:, cb, 14 * h:14 * h + 14, 0:28],
                        pt[:].rearrange("p (u v) -> p u v", v=28),
                        RELU, bias=neg_thr[:])

                conv1_fwd(x_pad, c1_init)
                c_cur, c_pre = c1A, c1B
                for it, beta in enumerate(BETAS):
                    if it == 0:
                        nc.vector.tensor_copy(a1[:, :, 0:28, 0:28],
                                              int1(c_cur))
                    else:
                        nc.vector.tensor_sub(dtmp[:, :, :, 0:28],
                                             int1(c_cur), int1(c_pre))
                        for kb in range(2):
                            nc.vector.affine_then_add(
                                a1[:, kb, 0:28, 0:28], dtmp[:, kb, :, 0:28],
                                c_cur[:, kb, 0:28, 0:28],
                                scale=float(beta), bias=0.0)
                    c_cur, c_pre = c_pre, c_cur  # c_cur now holds c_{k-2}

                    def r1_sub(cls, h, pt):
                        ey, ex = cls
                        sl = (slice(None),
                              slice(28 * h + ey + 1, 28 * h + ey + 29, 2),
                              slice(ex + 1, ex + 57, 2))
                        nc.vector.tensor_sub(
                            r1[sl], x_pad[sl],
                            pt[:].rearrange("p (u v) -> p u v", v=28))

                    conv1_t(a1, r1_sub)

                    def c1_step(cb, h, pt):
                        nc.vector.tensor_add(
                            c_cur[:, cb, 14 * h:14 * h + 14, 0:28],
                            a1[:, cb, 14 * h:14 * h + 14, 0:28],
                            pt[:].rearrange("p (u v) -> p u v", v=28))

                    conv1_fwd(r1, c1_step)
                    relu_into(c_cur, int1(c_cur))
                c1_fin = c_cur

                # BN1 -> x2 (padded interior)
                for kb in range(2):
                    nc.scalar.activation(
                        x2[:, kb, 1:29, 1:29], c1_fin[:, kb, 0:28, 0:28],
                        IDENT, bias=bn["bn1t"][:, kb:kb + 1],
                        scale=bn["bn1s"][:, kb:kb + 1])

                # ---------- block 2 (stride-1 FISTA on x2) ----------
                def c2_init(cb, h, pt):
                    nc.scalar.activation(
                        c2A[:, cb, 14 * h + 1:14 * h + 15, 1:29],
                        pt[:].rearrange("p (u v) -> p u v", v=28),
                        RELU, bias=neg_thr[:])

                conv2_fwd(x2, c2_init)
                c_cur, c_pre = c2A, c2B
                for it, beta in enumerate(BETAS):
                    if it == 0:
                        nc.vector.tensor_copy(a2[:, :, 1:29, 1:29],
                                              int2(c_cur))
                    else:
                        nc.vector.tensor_sub(dtmp[:, :, :, 0:28],
                                             int2(c_cur), int2(c_pre))
                        for kb in range(2):
                            nc.vector.affine_then_add(
                                a2[:, kb, 1:29, 1:29], dtmp[:, kb, :, 0:28],
                                c_cur[:, kb, 1:29, 1:29],
                                scale=float(beta), bias=0.0)
                    c_cur, c_pre = c_pre, c_cur

                    def r2_sub(cb, h, pt):
                        sl = (slice(None), cb,
                              slice(14 * h + 1, 14 * h + 15), slice(1, 29))
                        nc.vector.tensor_sub(
                            r2[sl], x2[sl],
                            pt[:].rearrange("p (u v) -> p u v", v=28))

                    conv2_t(a2, r2_sub)

                    def c2_step(cb, h, pt):
                        nc.vector.tensor_add(
                            c_cur[:, cb, 14 * h + 1:14 * h + 15, 1:29],
                            a2[:, cb, 14 * h + 1:14 * h + 15, 1:29],
                            pt[:].rearrange("p (u v) -> p u v", v=28))

                    conv2_fwd(r2, c2_step)
                    relu_into(c_cur, int2(c_cur))
                c2_fin = c_cur

                # ---------- shortcut + combine ----------
                o_sb = outp.tile([128, 2, 784], F32, name="o_sb", tag="osb")
                for kb in range(2):
                    nc.scalar.activation(
                        hb[:, kb].rearrange("p (u v) -> p u v", v=28),
                        c2_fin[:, kb, 1:29, 1:29],
                        IDENT, bias=bn["bn2t"][:, kb:kb + 1],
                        scale=bn["bn2s"][:, kb:kb + 1])
                for cb in range(2):
                    for h in range(2):
                        pt = ps_tile()
                        rhs = x_pad[:, 28 * h + 1: 28 * h + 29: 2, 1:57:2]
                        mm(pt, wsc[:, cb * 128:(cb + 1) * 128], rhs,
                           True, True)
                        nc.vector.affine_then_add(
                            o_sb[:, cb, 392 * h:392 * (h + 1)], pt[:],
                            hb[:, cb, 392 * h:392 * (h + 1)],
                            scale=bn["bnscs"][:, cb:cb + 1],
                            bias=bn["bnsct"][:, cb:cb + 1])
                nc.scalar.activation(o_sb[:], o_sb[:], RELU, bias=0.0)
                nc.sync.dma_start(
                    out=out_d.ap()[s].rearrange("(b p) h w -> p b (h w)",
                                                p=128),
                    in_=o_sb[:])

    nc.compile()
    return nc


def _np_dtype(cdt):
    return ml_dtypes.bfloat16 if cdt == BF16 else np.float32


def _prep_inputs(inputs, cdt):
    """Host-side weight prep + batch sharding. Returns in_maps (list of 8)."""
    npdt = _np_dtype(cdt)
    f32 = np.float32

    def norm(W):
        W = np.asarray(W, f32)
        n = np.sqrt((W * W).sum(axis=(1, 2, 3), keepdims=True))
        return W / (n + 1e-12)

    W1n = norm(inputs["W1"])
    W2n = norm(inputs["W2"])
    w1f = np.ascontiguousarray(
        (MU * W1n).transpose(1, 2, 3, 0).reshape(128, 9, 256)).astype(npdt)
    w1t = np.ascontiguousarray(
        W1n.reshape(2, 128, 128, 9).transpose(1, 0, 3, 2)).astype(npdt)
    w2f = np.ascontiguousarray(
        (MU * W2n).transpose(1, 2, 3, 0).reshape(2, 128, 9, 256)
        .transpose(1, 0, 2, 3)).astype(npdt)
    w2t = np.ascontiguousarray(
        W2n.reshape(2, 128, 256, 9).transpose(1, 0, 3, 2)).astype(npdt)
    wsc = np.ascontiguousarray(
        np.asarray(inputs["Wsc"], f32)[:, :, 0, 0].T).astype(npdt)

    def fold(pfx):
        g = np.asarray(inputs[pfx + "_g"], f32)
        b = np.asarray(inputs[pfx + "_b"], f32)
        m = np.asarray(inputs[pfx + "_m"], f32)
        v = np.asarray(inputs[pfx + "_v"], f32)
        s = g / np.sqrt(v + BN_EPS)
        t = b - m * s
        # [256] -> [128, 2] with [p, kb] = vec[kb*128 + p]
        return (np.ascontiguousarray(s.reshape(2, 128).T),
                np.ascontiguousarray(t.reshape(2, 128).T))

    bn1s, bn1t = fold("bn1")
    bn2s, bn2t = fold("bn2")
    bnscs, bnsct = fold("bnsc")

    x = np.asarray(inputs["x"], f32).astype(npdt)
    shared = dict(w1f=w1f, w1t=w1t, w2f=w2f, w2t=w2t, wsc=wsc,
                  bn1s=bn1s, bn1t=bn1t, bn2s=bn2s, bn2t=bn2t,
                  bnscs=bnscs, bnsct=bnsct)
    in_maps = []
    for c in range(N_CORES):
        m = dict(shared)
        m["x"] = np.ascontiguousarray(x[c * NS:(c + 1) * NS])
        in_maps.append(m)
    return in_maps


def _get_program(cdt):
    key = cdt.name
    if key not in _PROGRAM_CACHE:
        t0 = time.time()
        _PROGRAM_CACHE[key] = _build_program(cdt)
        KERNEL_STATS["build_s"] = time.time() - t0
    return _PROGRAM_CACHE[key]


def kernel(**inputs) -> np.ndarray:
    cdt = BF16 if os.environ.get("BASS_CDT", "bf16") == "bf16" else F32
    nc = _get_program(cdt)
    in_maps = _prep_inputs(inputs, cdt)
    t0 = time.time()
    res = run_bass_kernel_spmd(nc, in_maps, core_ids=list(range(N_CORES)))
    KERNEL_STATS["exec_s"] = time.time() - t0
    out = np.concatenate([res.results[c]["out"] for c in range(N_CORES)],
                         axis=0)
    return out


# revision 15
# speedup vs baseline: 64.4758x; 64.4758x over previous
# Trainium2 Bass kernel for nn_BasicBlock (FISTA sparse-coding BasicBlock).
#
# Data-parallel over batch: 32 samples -> 8 NeuronCores x 4 samples.
# Per sample, everything stays in SBUF; convs are 9-tap accumulating PE
# matmuls (channel blocks of 128 on partitions, 392-pixel N tiles);
# conv-transpose is computed via output-parity-class gather (9 taps total
# across 4 classes -> no zero-insertion waste). FISTA elementwise work runs
# on DVE/ACT. Dictionary normalization, MU scaling, weight transposes and
# BN folds are host-side numpy prep.
#
# Self-contained: hardcodes shapes from the problem spec.
import os
import sys
import time

sys.path.insert(0, "/opt/trn_rl_repo")

import numpy as np
import ml_dtypes

import concourse.bass as bass  # noqa: F401  (bass types referenced via bacc)
import concourse.mybir as mybir
from concourse import bacc
from concourse.bass_utils import run_bass_kernel_spmd
from concourse.tile import TileContext
from contextlib import ExitStack

F32 = mybir.dt.float32
BF16 = mybir.dt.bfloat16

MU = 0.1
LMBD = 0.1
N_STEPS = 4
BN_EPS = 1e-5
N_CORES = 8
NS = 4  # samples per core

RELU = mybir.ActivationFunctionType.Relu
IDENT = mybir.ActivationFunctionType.Identity

# FISTA momentum coefficients (matches reference's python-float t sequence)
BETAS = []
_t = 1.0
for _ in range(N_STEPS - 1):
    _tn = (1.0 + float(np.sqrt(1.0 + 4.0 * _t * _t))) / 2.0
    BETAS.append((_t - 1.0) / _tn)
    _t = _tn

# conv_t stride-2 parity classes: (ey, ex) -> [(dy, dx, du, dv)]
CT_CLASSES = [
    ((0, 0), [(1, 1, 0, 0)]),
    ((0, 1), [(1, 0, 0, 1), (1, 2, 0, 0)]),
    ((1, 0), [(0, 1, 1, 0), (2, 1, 0, 0)]),
    ((1, 1), [(0, 0, 1, 1), (0, 2, 1, 0), (2, 0, 0, 1), (2, 2, 0, 0)]),
]

KERNEL_STATS = {}
_PROGRAM_CACHE = {}


def _build_program(cdt):
    """Build + compile the per-core Bass program. cdt: matmul operand dtype."""
    nc = bacc.Bacc("TRN2", num_devices=1, debug=False)

    # x pre-split on host into padded-domain parity planes:
    # k=0: x[0::2,0::2] -> tile(1,1)[0:28,0:28]; k=1: x[0::2,1::2] ->
    # (1,0)[0:28,1:29]; k=2: x[1::2,0::2] -> (0,1)[1:29,0:28];
    # k=3: x[1::2,1::2] -> (0,0)[1:29,1:29]
    x_d = nc.dram_tensor("x", [NS, 4, 128, 28, 28], cdt,
                         kind="ExternalInput")
    w1f_d = nc.dram_tensor("w1f", [128, 9, 256], cdt, kind="ExternalInput")
    w1t_d = nc.dram_tensor("w1t", [128, 2, 9, 128], cdt, kind="ExternalInput")
    w2f_d = nc.dram_tensor("w2f", [128, 2, 9, 256], cdt, kind="ExternalInput")
    w2t_d = nc.dram_tensor("w2t", [128, 2, 9, 256], cdt, kind="ExternalInput")
    wsc_d = nc.dram_tensor("wsc", [128, 256], cdt, kind="ExternalInput")
    bn_d = {}
    for nm in ("bn1s", "bn1t", "bn2s", "bn2t", "bnscs", "bnsct"):
        bn_d[nm] = nc.dram_tensor(nm, [128, 2], F32, kind="ExternalInput")
    out_d = nc.dram_tensor("out", [NS, 256, 28, 28], F32, kind="ExternalOutput")

    with TileContext(nc) as tc:
        with ExitStack() as es:
            consts = es.enter_context(tc.tile_pool(name="consts", bufs=1))
            state = es.enter_context(tc.tile_pool(name="state", bufs=1))
            xin = es.enter_context(tc.tile_pool(name="xin", bufs=2))
            outp = es.enter_context(tc.tile_pool(name="outp", bufs=2))
            psum = es.enter_context(tc.tile_pool(name="psum", bufs=8, space="PSUM"))

            # ---- constants ----
            w1f = consts.tile([128, 9, 256], cdt)
            w1t = consts.tile([128, 2, 9, 128], cdt)
            w2f = consts.tile([128, 2, 9, 256], cdt)
            w2t = consts.tile([128, 2, 9, 256], cdt)
            wsc = consts.tile([128, 256], cdt)
            nc.sync.dma_start(out=w1f[:], in_=w1f_d.ap())
            nc.sync.dma_start(out=w1t[:], in_=w1t_d.ap())
            nc.sync.dma_start(out=w2f[:], in_=w2f_d.ap())
            nc.sync.dma_start(out=w2t[:], in_=w2t_d.ap())
            nc.sync.dma_start(out=wsc[:], in_=wsc_d.ap())
            bn = {}
            for nm in bn_d:
                bn[nm] = consts.tile([128, 2], F32, name=nm)
                nc.sync.dma_start(out=bn[nm][:], in_=bn_d[nm].ap())
            neg_thr = consts.tile([128, 1], F32)
            nc.vector.memset(neg_thr[:], -LMBD * MU)

            # ---- persistent per-sample state (interiors rewritten per
            # sample; borders stay zero from this one-time memset).
            # Two lanes (sample parity) so consecutive samples don't
            # serialize on shared buffers; f32 mode uses one lane (SBUF).
            n_lanes = 2 if cdt == BF16 else 1
            lanes = []
            for ln in range(n_lanes):
                st = {}
                for py in range(2):
                    for px in range(2):
                        st[f"r1_{py}{px}"] = state.tile(
                            [128, 29, 29], cdt, name=f"r1_{py}{px}_{ln}")
                st["a1"] = state.tile([128, 2, 29, 29], cdt, name=f"a1_{ln}")
                st["c1A"] = state.tile([128, 2, 29, 29], F32, name=f"c1A_{ln}")
                st["c1B"] = state.tile([128, 2, 29, 29], F32, name=f"c1B_{ln}")
                st["x2"] = state.tile([128, 2, 30, 30], cdt, name=f"x2_{ln}")
                st["r2"] = state.tile([128, 2, 30, 30], cdt, name=f"r2_{ln}")
                st["a2"] = state.tile([128, 2, 30, 30], cdt, name=f"a2_{ln}")
                st["c2A"] = state.tile([128, 2, 30, 30], F32, name=f"c2A_{ln}")
                st["c2B"] = state.tile([128, 2, 30, 30], F32, name=f"c2B_{ln}")
                st["hb"] = state.tile([128, 2, 784], F32, name=f"hb_{ln}")
                st["dtmp"] = state.tile([128, 2, 28, 29], F32,
                                        name=f"dtmp_{ln}")
                for k in ("r1_00", "r1_01", "r1_10", "r1_11", "a1",
                          "c1A", "c1B", "x2", "r2", "a2", "c2A", "c2B"):
                    nc.gpsimd.memset(st[k][:], 0.0)
                lanes.append(st)
            # fence: all init DMAs/memsets complete before any compute
            # (first-sample data race observed on HW without this)
            tc.strict_bb_all_engine_barrier()

            def ps_tile():
                return psum.tile([128, 392], F32, name="pt", tag="ps")

            def mm(pt, lhsT, rhs, start, stop):
                nc.tensor.matmul(
                    pt[:].rearrange("p (u v) -> p u v", v=28),
                    lhsT, rhs, start=start, stop=stop)

            # ---- conv emitters (all produce 4 or 8 psum tiles, then run
            # `consume(cb_or_class, h, pt)`) ----
            def conv1_fwd(srcP, consume):
                # stride-2 3x3 conv via parity tiles: tap (dy,dx) reads
                # parity tile (dy%2, dx%2) with unit-stride APs.
                # h innermost so both halves share one weight load.
                for cb in range(2):
                    pts = [ps_tile(), ps_tile()]
                    for tap in range(9):
                        dy, dx = tap // 3, tap % 3
                        t_ = srcP[(dy % 2, dx % 2)]
                        for h in range(2):
                            r0 = 14 * h + dy // 2
                            rhs = t_[:, r0: r0 + 14,
                                     dx // 2: dx // 2 + 28]
                            mm(pts[h], w1f[:, tap, cb * 128:(cb + 1) * 128],
                               rhs, tap == 0, tap == 8)
                    for h in range(2):
                        consume(cb, h, pts[h])

            def conv1_t(src, consume):
                # stride-2 conv-transpose, 256 -> 128, src [128,2,29,29]
                # h innermost so both halves share one weight load.
                for (ey, ex), taps in CT_CLASSES:
                    pts = [ps_tile(), ps_tile()]
                    n = 2 * len(taps)
                    i = 0
                    for (dy, dx, du, dv) in taps:
                        tap = dy * 3 + dx
                        for kb in range(2):
                            for h in range(2):
                                rhs = src[:, kb, du + 14 * h: du + 14 * h + 14,
                                          dv: dv + 28]
                                mm(pts[h], w1t[:, kb, tap, :], rhs,
                                   i == 0, i == n - 1)
                            i += 1
                    for h in range(2):
                        consume((ey, ex), h, pts[h])

            def conv2_fwd(src, consume):
                # stride-1 3x3 conv, 256 -> 256, src [128,2,30,30] padded
                for cb in range(2):
                    pts = [ps_tile(), ps_tile()]
                    for i in range(18):
                        kb, tap = i // 9, i % 9
                        dy, dx = tap // 3, tap % 3
                        for h in range(2):
                            rhs = src[:, kb, 14 * h + dy: 14 * h + dy + 14,
                                      dx: dx + 28]
                            mm(pts[h], w2f[:, kb, tap, cb * 128:(cb + 1) * 128],
                               rhs, i == 0, i == 17)
                    for h in range(2):
                        consume(cb, h, pts[h])

            def conv2_t(src, consume):
                # stride-1 conv-transpose (flipped taps), src [128,2,30,30]
                for cb in range(2):
                    pts = [ps_tile(), ps_tile()]
                    for i in range(18):
                        kb, tap = i // 9, i % 9
                        dy, dx = tap // 3, tap % 3
                        for h in range(2):
                            rhs = src[:, kb, (2 - dy) + 14 * h:
                                      (2 - dy) + 14 * h + 14,
                                      (2 - dx): (2 - dx) + 28]
                            mm(pts[h], w2t[:, kb, tap, cb * 128:(cb + 1) * 128],
                               rhs, i == 0, i == 17)
                    for h in range(2):
                        consume(cb, h, pts[h])

            def int1(c):  # interior of a block-1 c tile ([128,2,28,28])
                return c[:, :, 0:28, 0:28]

            def int2(c):  # interior of a block-2 30x30 tile
                return c[:, :, 1:29, 1:29]

            def relu_into(c_t, interior):
                # c = relu(c - lmbd*mu) in place over the interior view
                nc.scalar.activation(interior, interior, RELU, bias=neg_thr[:])
                _ = c_t  # (kept for readability at call sites)

            # ================= per-sample program =================
            # Each sample's work is split into phases; two parity lanes are
            # emitted interleaved so the PE always has the other lane's
            # matmuls to chew on while one lane runs its elementwise chain
            # (relu/momentum) between convs.
            def sample_phases(s, st):
                """Returns list of phase-emitter closures for sample s."""
                r1P = {(py, px): st[f"r1_{py}{px}"]
                       for py in range(2) for px in range(2)}
                a1, c1A, c1B = st["a1"], st["c1A"], st["c1B"]
                x2, r2, a2 = st["x2"], st["r2"], st["a2"]
                c2A, c2B, hb, dtmp = (st["c2A"], st["c2B"], st["hb"],
                                      st["dtmp"])
                ctx = {}
                phases = []

                def ph_load():
                    # x in parity layout: xP[(py,px)][u,v] = x_pad[2u+py,
                    # 2v+px] (padded-domain coords, +1 pad origin)
                    xP = {}
                    for py in range(2):
                        for px in range(2):
                            xP[(py, px)] = xin.tile(
                                [128, 29, 29], cdt, name=f"xp{py}{px}",
                                tag=f"xp{py}{px}")
                    ctx["xP"] = xP
                    nc.gpsimd.memset(xP[(0, 0)][:, 0, :], 0.0)
                    nc.gpsimd.memset(xP[(0, 0)][:, :, 0], 0.0)
                    nc.gpsimd.memset(xP[(0, 1)][:, 0, :], 0.0)
                    nc.gpsimd.memset(xP[(1, 0)][:, :, 0], 0.0)
                    xs = x_d.ap()[s]
                    nc.sync.dma_start(out=xP[(1, 1)][:, 0:28, 0:28],
                                      in_=xs[0])
                    nc.sync.dma_start(out=xP[(1, 0)][:, 0:28, 1:29],
                                      in_=xs[1])
                    nc.sync.dma_start(out=xP[(0, 1)][:, 1:29, 0:28],
                                      in_=xs[2])
                    nc.sync.dma_start(out=xP[(0, 0)][:, 1:29, 1:29],
                                      in_=xs[3])
                phases.append(ph_load)

                def ph_init1():
                    def c1_init(cb, h, pt):
                        nc.scalar.activation(
                            c1A[:, cb, 14 * h:14 * h + 14, 0:28],
                            pt[:].rearrange("p (u v) -> p u v", v=28),
                            RELU, bias=neg_thr[:])
                    conv1_fwd(ctx["xP"], c1_init)
                    ctx["c_cur"], ctx["c_pre"] = c1A, c1B
                phases.append(ph_init1)

                for it_, beta_ in enumerate(BETAS):
                    def ph_b1_ct(it=it_, beta=beta_):
                        c_cur, c_pre = ctx["c_cur"], ctx["c_pre"]
                        if it == 0:
                            nc.vector.tensor_copy(a1[:, :, 0:28, 0:28],
                                                  int1(c_cur))
                        else:
                            nc.vector.tensor_sub(dtmp[:, :, :, 0:28],
                                                 int1(c_cur), int1(c_pre))
                            for kb in range(2):
                                nc.vector.affine_then_add(
                                    a1[:, kb, 0:28, 0:28],
                                    dtmp[:, kb, :, 0:28],
                                    c_cur[:, kb, 0:28, 0:28],
                                    scale=float(beta), bias=0.0)
                        ctx["c_cur"], ctx["c_pre"] = c_pre, c_cur

                        xP = ctx["xP"]

                        def r1_sub(cls, h, pt):
                            # class (ey,ex) lands in parity tile
                            # ((ey+1)%2, (ex+1)%2) at offset (ey+1)//2
                            ey, ex = cls
                            py, px = (ey + 1) % 2, (ex + 1) % 2
                            ro, co = (ey + 1) // 2, (ex + 1) // 2
                            sl = (slice(None),
                                  slice(ro + 14 * h, ro + 14 * h + 14),
                                  slice(co, co + 28))
                            nc.vector.tensor_sub(
                                r1P[(py, px)][sl], xP[(py, px)][sl],
                                pt[:].rearrange("p (u v) -> p u v", v=28))
                        conv1_t(a1, r1_sub)
                    phases.append(ph_b1_ct)

                    def ph_b1_cf(it=it_):
                        c_cur = ctx["c_cur"]

                        def c1_step(cb, h, pt):
                            nc.vector.tensor_add(
                                c_cur[:, cb, 14 * h:14 * h + 14, 0:28],
                                a1[:, cb, 14 * h:14 * h + 14, 0:28],
                                pt[:].rearrange("p (u v) -> p u v", v=28))
                        conv1_fwd(r1P, c1_step)
                        relu_into(c_cur, int1(c_cur))
                    phases.append(ph_b1_cf)

                def ph_bn1_init2():
                    c1_fin = ctx["c_cur"]
                    for kb in range(2):
                        nc.scalar.activation(
                            x2[:, kb, 1:29, 1:29], c1_fin[:, kb, 0:28, 0:28],
                            IDENT, bias=bn["bn1t"][:, kb:kb + 1],
                            scale=bn["bn1s"][:, kb:kb + 1])

                    def c2_init(cb, h, pt):
                        nc.scalar.activation(
                            c2A[:, cb, 14 * h + 1:14 * h + 15, 1:29],
                            pt[:].rearrange("p (u v) -> p u v", v=28),
                            RELU, bias=neg_thr[:])
                    conv2_fwd(x2, c2_init)
                    ctx["c_cur"], ctx["c_pre"] = c2A, c2B
                phases.append(ph_bn1_init2)

                for it_, beta_ in enumerate(BETAS):
                    def ph_b2_ct(it=it_, beta=beta_):
                        c_cur, c_pre = ctx["c_cur"], ctx["c_pre"]
                        if it == 0:
                            nc.vector.tensor_copy(a2[:, :, 1:29, 1:29],
                                                  int2(c_cur))
                        else:
                            nc.vector.tensor_sub(dtmp[:, :, :, 0:28],
                                                 int2(c_cur), int2(c_pre))
                            for kb in range(2):
                                nc.vector.affine_then_add(
                                    a2[:, kb, 1:29, 1:29],
                                    dtmp[:, kb, :, 0:28],
                                    c_cur[:, kb, 1:29, 1:29],
                                    scale=float(beta), bias=0.0)
                        ctx["c_cur"], ctx["c_pre"] = c_pre, c_cur

                        def r2_sub(cb, h, pt):
                            sl = (slice(None), cb,
                                  slice(14 * h + 1, 14 * h + 15),
                                  slice(1, 29))
                            nc.vector.tensor_sub(
                                r2[sl], x2[sl],
                                pt[:].rearrange("p (u v) -> p u v", v=28))
                        conv2_t(a2, r2_sub)
                    phases.append(ph_b2_ct)

                    def ph_b2_cf(it=it_):
                        c_cur = ctx["c_cur"]

                        def c2_step(cb, h, pt):
                            nc.vector.tensor_add(
                                c_cur[:, cb, 14 * h + 1:14 * h + 15, 1:29],
                                a2[:, cb, 14 * h + 1:14 * h + 15, 1:29],
                                pt[:].rearrange("p (u v) -> p u v", v=28))
                        conv2_fwd(r2, c2_step)
                        relu_into(c_cur, int2(c_cur))
                    phases.append(ph_b2_cf)

                def ph_out():
                    c2_fin = ctx["c_cur"]
                    xP = ctx["xP"]
                    o_sb = outp.tile([128, 2, 784], F32, name="o_sb",
                                     tag="osb")
                    for kb in range(2):
                        nc.scalar.activation(
                            hb[:, kb].rearrange("p (u v) -> p u v", v=28),
                            c2_fin[:, kb, 1:29, 1:29],
                            IDENT, bias=bn["bn2t"][:, kb:kb + 1],
                            scale=bn["bn2s"][:, kb:kb + 1])
                    for cb in range(2):
                        for h in range(2):
                            pt = ps_tile()
                            rhs = xP[(1, 1)][:, 14 * h: 14 * h + 14,
                                             0:28]
                            mm(pt, wsc[:, cb * 128:(cb + 1) * 128], rhs,
                               True, True)
                            nc.vector.affine_then_add(
                                o_sb[:, cb, 392 * h:392 * (h + 1)], pt[:],
                                hb[:, cb, 392 * h:392 * (h + 1)],
                                scale=bn["bnscs"][:, cb:cb + 1],
                                bias=bn["bnsct"][:, cb:cb + 1])
                    nc.scalar.activation(o_sb[:], o_sb[:], RELU, bias=0.0)
                    nc.sync.dma_start(
                        out=out_d.ap()[s].rearrange(
                            "(b p) h w -> p b (h w)", p=128),
                        in_=o_sb[:])
                phases.append(ph_out)
                return phases

            reps = int(os.environ.get("BASS_REPS", "1"))
            order = [i % NS for i in range(NS * reps)]
            if n_lanes == 2:
                for base in range(0, len(order), 2):
                    pair = order[base:base + 2]
                    plists = [sample_phases(s, lanes[j])
                              for j, s in enumerate(pair)]
                    # lane 1 trails lane 0 by one phase so its conv fills
                    # lane 0's elementwise chain
                    for k in range(len(plists[0])):
                        for pl in plists:
                            if k < len(pl):
                                pl[k]()
            else:
                for s in order:
                    for ph in sample_phases(s, lanes[0]):
                        ph()

    nc.compile()
    return nc


def _np_dtype(cdt):
    return ml_dtypes.bfloat16 if cdt == BF16 else np.float32


def _prep_inputs(inputs, cdt):
    """Host-side weight prep + batch sharding. Returns in_maps (list of 8)."""
    npdt = _np_dtype(cdt)
    f32 = np.float32

    def norm(W):
        W = np.asarray(W, f32)
        n = np.sqrt((W * W).sum(axis=(1, 2, 3), keepdims=True))
        return W / (n + 1e-12)

    W1n = norm(inputs["W1"])
    W2n = norm(inputs["W2"])
    w1f = np.ascontiguousarray(
        (MU * W1n).transpose(1, 2, 3, 0).reshape(128, 9, 256)).astype(npdt)
    w1t = np.ascontiguousarray(
        W1n.reshape(2, 128, 128, 9).transpose(1, 0, 3, 2)).astype(npdt)
    w2f = np.ascontiguousarray(
        (MU * W2n).transpose(1, 2, 3, 0).reshape(2, 128, 9, 256)
        .transpose(1, 0, 2, 3)).astype(npdt)
    w2t = np.ascontiguousarray(
        W2n.reshape(2, 128, 256, 9).transpose(1, 0, 3, 2)).astype(npdt)
    wsc = np.ascontiguousarray(
        np.asarray(inputs["Wsc"], f32)[:, :, 0, 0].T).astype(npdt)

    def fold(pfx):
        g = np.asarray(inputs[pfx + "_g"], f32)
        b = np.asarray(inputs[pfx + "_b"], f32)
        m = np.asarray(inputs[pfx + "_m"], f32)
        v = np.asarray(inputs[pfx + "_v"], f32)
        s = g / np.sqrt(v + BN_EPS)
        t = b - m * s
        # [256] -> [128, 2] with [p, kb] = vec[kb*128 + p]
        return (np.ascontiguousarray(s.reshape(2, 128).T),
                np.ascontiguousarray(t.reshape(2, 128).T))

    bn1s, bn1t = fold("bn1")
    bn2s, bn2t = fold("bn2")
    bnscs, bnsct = fold("bnsc")

    x = np.asarray(inputs["x"], f32).astype(npdt)
    # parity pre-split: [N, 4, 128, 28, 28]
    x = np.stack([x[:, :, 0::2, 0::2], x[:, :, 0::2, 1::2],
                  x[:, :, 1::2, 0::2], x[:, :, 1::2, 1::2]], axis=1)
    x = np.ascontiguousarray(x)
    shared = dict(w1f=w1f, w1t=w1t, w2f=w2f, w2t=w2t, wsc=wsc,
                  bn1s=bn1s, bn1t=bn1t, bn2s=bn2s, bn2t=bn2t,
                  bnscs=bnscs, bnsct=bnsct)
    in_maps = []
    for c in range(N_CORES):
        m = dict(shared)
        m["x"] = np.ascontiguousarray(x[c * NS:(c + 1) * NS])
        in_maps.append(m)
    return in_maps


def _get_program(cdt):
    key = cdt.name
    if key not in _PROGRAM_CACHE:
        t0 = time.time()
        _PROGRAM_CACHE[key] = _build_program(cdt)
        KERNEL_STATS["build_s"] = time.time() - t0
    return _PROGRAM_CACHE[key]


def kernel(**inputs) -> np.ndarray:
    cdt = BF16 if os.environ.get("BASS_CDT", "bf16") == "bf16" else F32
    nc = _get_program(cdt)
    in_maps = _prep_inputs(inputs, cdt)
    t0 = time.time()
    res = run_bass_kernel_spmd(nc, in_maps, core_ids=list(range(N_CORES)))
    KERNEL_STATS["exec_s"] = time.time() - t0
    out = np.concatenate([res.results[c]["out"] for c in range(N_CORES)],
                         axis=0)
    return out
